# revision 6
# baseline (speedup 1.0000x reference)
"""Cross-attention (B=4, C=256, H=W=64) on 8 TRN2 NeuronCores.

Sharding: data-parallel over batch (4) x sequence-parallel over query dim
(2 halves of n=4096), one (batch, half) pair per core. Full n x n attention
stays on-core; softmax rows (over keys j) are complete locally.

Per-core math, all layouts chosen so no large transpose is ever needed:
  Q[c,i]  = WqT.T @ x1_half + bq          (fp16 matmuls, DVE bias-add)
  K[c,j]  = WkT.T @ x2 + bk
  Vt[j,c] = x2_chunk.T @ WvT + bv         (V transposed "for free", bf16;
                                           bv folded here: softmax weights
                                           sum to 1, so (V+bv)@attn adds bv
                                           to the output exactly)
  St[j,i] = K_chunk.T @ Q                 (K chunk stationary, fp16, fp32 acc)
  Et      = exp(St - C0)                  (ACT, bf16 out; C0 global shift,
                                           cancels exactly in normalization;
                                           bf16 keeps fp32 exponent range)
  O[c,i] += Vt_chunk.T @ Et               (bf16, accumulated over j in PSUM)
  sumE[i] = ones.T @ (sum_chunks Et)      (DVE accumulate + matmul, fp32)
  out     = O * (1/sumE)                  (DVE reciprocal + mul, fp32)

The emission order software-pipelines the attention inner loop one step:
S[jc+1] enters the (strict FIFO) tensor queue BEFORE O[jc], so the PE works
on S[jc+1] while ACT computes exp[jc] instead of stalling behind O[jc]'s
dependency on it. ACT does nothing but the 64 exps (projection bias-adds
live on DVE), making exp latency the only ACT-side constraint. K/V
projection chunks are interleaved into the first i-block's attention stream
so projection PE work overlaps early exp ACT work.

fp16 carries the accuracy-critical S side (11-bit mantissa at full PE rate);
bf16 carries E (values up to e^79 need fp32-range exponents). All PSUM
accumulation is fp32; normalization is exact fp32.
reps>1 repeats the compute body inside one NEFF (timing harness only);
full_body additionally repeats the input DMAs each rep.
"""
import numpy as np

import concourse.bacc as bacc
import concourse.mybir as mybir
import concourse.tile as tile
from concourse.bass_utils import run_bass_kernel_spmd

B, C, H, W = 4, 256, 64, 64
N = H * W                 # 4096 keys per sample
NQ = N // 2               # 2048 queries per core
CC = C // 128             # 2 channel chunks
NJ = N // 128             # 32 key chunks
IB = 2                    # i-blocks of 1024 queries
IBS = NQ // IB            # 1024
C0 = 72.0                 # global softmax shift (see module docstring)

F32 = mybir.dt.float32
F16 = mybir.dt.float16
BF16 = mybir.dt.bfloat16
EXP = mybir.ActivationFunctionType.Exp

_CACHED = {}


def _build(reps=1, full_body=False):
    nc = bacc.Bacc()
    x1s = nc.dram_tensor("x1s", [C, NQ], F16, kind="ExternalInput")
    x2 = nc.dram_tensor("x2", [C, N], F16, kind="ExternalInput")
    wqT = nc.dram_tensor("wqT", [C, C], F16, kind="ExternalInput")
    wkT = nc.dram_tensor("wkT", [C, C], F16, kind="ExternalInput")
    wvT = nc.dram_tensor("wvT", [C, C], F16, kind="ExternalInput")
    bq = nc.dram_tensor("bq", [C, 1], F32, kind="ExternalInput")
    bk = nc.dram_tensor("bk", [C, 1], F32, kind="ExternalInput")
    bv = nc.dram_tensor("bv", [1, C], F32, kind="ExternalInput")
    out = nc.dram_tensor("o", [C, NQ], F32, kind="ExternalOutput")

    with tile.TileContext(nc) as tc:
        with (
            tc.tile_pool(name="singles", bufs=1) as singles,
            tc.tile_pool(name="epool", bufs=4) as epool,
            tc.tile_pool(name="accp", bufs=2) as accp,
            tc.tile_pool(name="ep1", bufs=2) as ep1,
            tc.tile_pool(name="ep2", bufs=2) as ep2,
            tc.tile_pool(name="ps_s", bufs=2, space="PSUM") as ps_s,  # 2x 2-bank
            tc.tile_pool(name="ps_o", bufs=1, space="PSUM") as ps_o,  # 4 banks
        ):
            # ---------------- tiles + constants ----------------
            w_q = singles.tile([128, CC, CC, 128], F16)
            w_k = singles.tile([128, CC, CC, 128], F16)
            w_v = singles.tile([128, CC, CC, 128], F16)
            b_q = singles.tile([128, CC, 1], F32)
            b_k = singles.tile([128, CC, 1], F32)
            bv_sb = singles.tile([1, C], F32)
            x1_t = singles.tile([128, CC, NQ], F16)
            x2_t = singles.tile([128, CC, N], F16)

            ones_jm = singles.tile([128, 128], F32)
            nc.vector.memset(ones_jm, 1.0)
            negc0 = singles.tile([128, 1], F32)
            nc.vector.memset(negc0, -C0)

            def issue_loads():
                nc.sync.dma_start(
                    out=w_q, in_=wqT.ap().rearrange("(ci k) (co m) -> k ci co m", k=128, m=128)
                )
                for t, d in ((b_q, bq), (b_k, bk)):
                    nc.sync.dma_start(out=t, in_=d.ap().rearrange("(cc c) x -> c cc x", c=128))
                nc.sync.dma_start(out=bv_sb, in_=bv.ap())
                # chunked input loads so projections can start on partial data
                x1_ap = x1s.ap().rearrange("(cc c) n -> c cc n", c=128)
                for nb in range(NQ // 1024):
                    sl = slice(nb * 1024, (nb + 1) * 1024)
                    nc.sync.dma_start(out=x1_t[:, :, sl], in_=x1_ap[:, :, sl])
                for t, d in ((w_k, wkT), (w_v, wvT)):
                    nc.sync.dma_start(
                        out=t, in_=d.ap().rearrange("(ci k) (co m) -> k ci co m", k=128, m=128)
                    )
                x2_ap = x2.ap().rearrange("(cc c) n -> c cc n", c=128)
                for nb in range(N // 512):
                    sl = slice(nb * 512, (nb + 1) * 512)
                    nc.sync.dma_start(out=x2_t[:, :, sl], in_=x2_ap[:, :, sl])

            if not full_body:
                issue_loads()

            # bv broadcast across partitions: [128, C] = ones-column x bv_row
            bvv = singles.tile([128, C], F32)

            def compute_bvv():
                bvv_ps = ps_s.tile([128, C], F32, tag="s", name="bvv_ps")
                nc.tensor.matmul(bvv_ps, lhsT=ones_jm[0:1, :], rhs=bv_sb, start=True, stop=True)
                nc.vector.tensor_copy(bvv, bvv_ps)

            if not full_body:
                compute_bvv()

            q_t = singles.tile([128, CC, NQ], F16)
            k_t = singles.tile([128, CC, N], F16)
            v_t = singles.tile([128, NJ, C], BF16)

            def emit_kv_proj(nb):
                sl = slice(nb * 512, (nb + 1) * 512)
                for co in range(CC):
                    ps = ps_s.tile([128, 512], F32, tag="s", name="ps")
                    for ci in range(CC):
                        nc.tensor.matmul(
                            ps, lhsT=w_k[:, ci, co, :], rhs=x2_t[:, ci, sl],
                            start=(ci == 0), stop=(ci == CC - 1),
                        )
                    nc.vector.tensor_scalar_add(k_t[:, co, sl], ps, b_k[:, co, :])
                for jp in range(nb * 2, nb * 2 + 2):
                    ps = ps_s.tile([128, 2, C], F32, tag="s", name="ps")
                    for u in range(2):
                        jc = jp * 2 + u
                        jsl = slice(jc * 128, (jc + 1) * 128)
                        for ci in range(CC):
                            nc.tensor.matmul(
                                ps[:, u, :], lhsT=x2_t[:, ci, jsl],
                                rhs=w_v[:, ci, :, :],
                                start=(ci == 0), stop=(ci == CC - 1),
                            )
                    nc.vector.tensor_add(
                        v_t[:, jp * 2:jp * 2 + 2, :], ps,
                        bvv.rearrange("p (a c) -> p a c", a=1).broadcast_to([128, 2, C]),
                    )

            for _rep in range(reps):
                if full_body:
                    issue_loads()
                    compute_bvv()
                # Q projection upfront (attention needs it immediately)
                for nb in range(NQ // 512):
                    sl = slice(nb * 512, (nb + 1) * 512)
                    for co in range(CC):
                        ps = ps_s.tile([128, 512], F32, tag="s", name="ps")
                        for ci in range(CC):
                            nc.tensor.matmul(
                                ps, lhsT=w_q[:, ci, co, :], rhs=x1_t[:, ci, sl],
                                start=(ci == 0), stop=(ci == CC - 1),
                            )
                        nc.vector.tensor_scalar_add(q_t[:, co, sl], ps, b_q[:, co, :])

                # ------------- attention (K/V proj interleaved into ib 0) ----
                for ib in range(IB):
                    o_ps = [
                        ps_o.tile([128, 2, 512], F32, tag=f"o{cc}", name=f"ops{cc}")
                        for cc in range(CC)
                    ]
                    acc = accp.tile([128, IBS], F32, tag="acc")
                    prev_e = None

                    def emit_o_acc(jcp, e):
                        for cc in range(CC):
                            csl = slice(cc * 128, (cc + 1) * 128)
                            for h in range(2):
                                hsl = slice(h * 512, (h + 1) * 512)
                                nc.tensor.matmul(
                                    o_ps[cc][:, h, :], lhsT=v_t[:, jcp, csl],
                                    rhs=e[:, hsl],
                                    start=(jcp == 0), stop=(jcp == NJ - 1),
                                )
                        if jcp == 0:
                            nc.vector.tensor_copy(acc, e)
                        else:
                            nc.vector.tensor_add(acc, acc, e)

                    for jc in range(NJ):
                        if ib == 0 and jc % 4 == 0:
                            emit_kv_proj(jc // 4)
                        jsl = slice(jc * 128, (jc + 1) * 128)
                        s_ps = ps_s.tile([128, IBS], F32, tag="s", name="s_ps")
                        for ci in range(CC):
                            for h in range(2):
                                hsl = slice(h * 512, (h + 1) * 512)
                                qsl = slice(ib * IBS + h * 512, ib * IBS + (h + 1) * 512)
                                nc.tensor.matmul(
                                    s_ps[:, hsl], lhsT=k_t[:, ci, jsl], rhs=q_t[:, ci, qsl],
                                    start=(ci == 0), stop=(ci == CC - 1),
                                )
                        if prev_e is not None:
                            emit_o_acc(jc - 1, prev_e)
                        e_t = epool.tile([128, IBS], BF16, tag="e")
                        nc.scalar.activation(e_t, s_ps, EXP, bias=negc0, scale=1.0)
                        prev_e = e_t
                    emit_o_acc(NJ - 1, prev_e)

                    # ---- epilogue: normalize + store ----
                    # ones[128,128].T @ acc = sumE broadcast across partitions;
                    # reciprocal then runs 128 lanes wide, no extra broadcast
                    rb_sb = ep1.tile([128, 2, 512], F32, tag="rb")
                    for h in range(2):
                        hsl = slice(h * 512, (h + 1) * 512)
                        rs_ps = ps_s.tile([128, 512], F32, tag="s", name=f"rs{h}")
                        nc.tensor.matmul(rs_ps, lhsT=ones_jm, rhs=acc[:, hsl],
                                         start=True, stop=True)
                        nc.vector.reciprocal(rb_sb[:, h, :], rs_ps)
                    for cc in range(CC):
                        o1 = ep2.tile([128, 2, 512], F32, tag="o1")
                        nc.vector.tensor_mul(o1, o_ps[cc], rb_sb)
                        nc.sync.dma_start(
                            out=out.ap()[cc * 128:(cc + 1) * 128,
                                         ib * IBS:(ib + 1) * IBS]
                            .rearrange("c (a b) -> c a b", a=2),
                            in_=o1,
                        )
    nc.compile()
    return nc


def kernel(x1, x2, Wq, bq, Wk, bk, Wv, bv):
    x1 = np.ascontiguousarray(np.asarray(x1, dtype=np.float32)).reshape(B, C, N).astype(np.float16)
    x2 = np.ascontiguousarray(np.asarray(x2, dtype=np.float32)).reshape(B, C, N).astype(np.float16)
    wqT = np.ascontiguousarray(np.asarray(Wq, dtype=np.float32).T).astype(np.float16)
    wkT = np.ascontiguousarray(np.asarray(Wk, dtype=np.float32).T).astype(np.float16)
    wvT = np.ascontiguousarray(np.asarray(Wv, dtype=np.float32).T).astype(np.float16)
    bq = np.asarray(bq, dtype=np.float32).reshape(C, 1)
    bk = np.asarray(bk, dtype=np.float32).reshape(C, 1)
    bv = np.asarray(bv, dtype=np.float32).reshape(1, C)

    if "nc" not in _CACHED:
        _CACHED["nc"] = _build()
    nc = _CACHED["nc"]

    in_maps = []
    for core in range(8):
        b, half = divmod(core, 2)
        in_maps.append({
            "x1s": np.ascontiguousarray(x1[b][:, half * NQ:(half + 1) * NQ]),
            "x2": x2[b],
            "wqT": wqT, "wkT": wkT, "wvT": wvT,
            "bq": bq, "bk": bk, "bv": bv,
        })
    res = run_bass_kernel_spmd(nc, in_maps, core_ids=list(range(8)))
    out = np.empty((B, C, N), dtype=np.float32)
    for core in range(8):
        b, half = divmod(core, 2)
        out[b][:, half * NQ:(half + 1) * NQ] = res.results[core]["o"]
    return out.reshape(B, C, H, W)


# revision 9
# speedup vs baseline: 1.1375x; 1.1375x over previous
"""Cross-attention (B=4, C=256, H=W=64) on 8 TRN2 NeuronCores.

Sharding: data-parallel over batch (4) x sequence-parallel over query dim
(2 halves of n=4096), one (batch, half) pair per core. Full n x n attention
stays on-core; softmax rows (over keys j) are complete locally.

Per-core math, all layouts chosen so no large transpose is ever needed:
  Q[c,i]  = WqT.T @ x1_half + bq          (fp16 matmuls, DVE bias-add)
  K[c,j]  = WkT.T @ x2 + bk
  Vt[j,c] = x2_chunk.T @ WvT + bv         (V transposed "for free", bf16;
                                           bv folded here: softmax weights
                                           sum to 1, so (V+bv)@attn adds bv
                                           to the output exactly)
  St[j,i] = K_chunk.T @ Q                 (K chunk stationary, fp16, fp32 acc)
  Et      = exp(St - C0)                  (ACT, bf16 out; C0 global shift,
                                           cancels exactly in normalization;
                                           bf16 keeps fp32 exponent range.
                                           Split into 512-col halves so the
                                           ACT latency per half (~720ns) is
                                           below the S half-block PE time --
                                           the S->exp->O chain then pipelines
                                           with zero PE gaps at h granularity)
  O[c,i] += Vt_chunk.T @ Et               (bf16, accumulated over j in PSUM)
  sumE[i] = ones.T @ (sum_chunks Et)      (DVE accumulate + matmul, fp32)
  out     = O * (1/sumE)                  (DVE reciprocal + mul, fp32)

ACT does nothing but the 128 exp halves (projection bias-adds live on DVE
as tensor_scalar_add), so the body is ACT-exp-bound: everything else (all
matmuls, DVE accumulation, DMA) hides underneath the exp stream.

fp16 carries the accuracy-critical S side (11-bit mantissa at full PE rate);
bf16 carries E (values up to e^79 need fp32-range exponents). All PSUM
accumulation is fp32; normalization is exact fp32.
reps>1 repeats the compute body inside one NEFF (timing harness only);
full_body additionally repeats the input DMAs each rep.
"""
import numpy as np

import concourse.bacc as bacc
import concourse.mybir as mybir
import concourse.tile as tile
from concourse.bass_utils import run_bass_kernel_spmd

B, C, H, W = 4, 256, 64, 64
N = H * W                 # 4096 keys per sample
NQ = N // 2               # 2048 queries per core
CC = C // 128             # 2 channel chunks
NJ = N // 128             # 32 key chunks
IB = 2                    # i-blocks of 1024 queries
IBS = NQ // IB            # 1024
C0 = 72.0                 # global softmax shift (see module docstring)

F32 = mybir.dt.float32
F16 = mybir.dt.float16
BF16 = mybir.dt.bfloat16
EXP = mybir.ActivationFunctionType.Exp

_CACHED = {}


def _build(reps=1, full_body=False):
    nc = bacc.Bacc()
    x1s = nc.dram_tensor("x1s", [C, NQ], F16, kind="ExternalInput")
    x2 = nc.dram_tensor("x2", [C, N], F16, kind="ExternalInput")
    wqT = nc.dram_tensor("wqT", [C, C], F16, kind="ExternalInput")
    wkT = nc.dram_tensor("wkT", [C, C], F16, kind="ExternalInput")
    wvT = nc.dram_tensor("wvT", [C, C], F16, kind="ExternalInput")
    bq = nc.dram_tensor("bq", [C, 1], F32, kind="ExternalInput")
    bk = nc.dram_tensor("bk", [C, 1], F32, kind="ExternalInput")
    bv = nc.dram_tensor("bv", [1, C], F32, kind="ExternalInput")
    out = nc.dram_tensor("o", [C, NQ], F32, kind="ExternalOutput")

    with tile.TileContext(nc) as tc:
        with (
            tc.tile_pool(name="singles", bufs=1) as singles,
            tc.tile_pool(name="epool", bufs=4) as epool,
            tc.tile_pool(name="accp", bufs=2) as accp,
            tc.tile_pool(name="ep1", bufs=2) as ep1,
            tc.tile_pool(name="ep2", bufs=2) as ep2,
            tc.tile_pool(name="ps_s", bufs=4, space="PSUM") as ps_s,  # 4x 1-bank
            tc.tile_pool(name="ps_o", bufs=1, space="PSUM") as ps_o,  # 4 banks
        ):
            # ---------------- tiles + constants ----------------
            w_q = singles.tile([128, CC, CC, 128], F16)
            w_k = singles.tile([128, CC, CC, 128], F16)
            w_v = singles.tile([128, CC, CC, 128], F16)
            b_q = singles.tile([128, CC, 1], F32)
            b_k = singles.tile([128, CC, 1], F32)
            bv_sb = singles.tile([1, C], F32)
            x1_t = singles.tile([128, CC, NQ], F16)
            x2_t = singles.tile([128, CC, N], F16)

            ones_jm = singles.tile([128, 128], F32)
            nc.vector.memset(ones_jm, 1.0)
            negc0 = singles.tile([128, 1], F32)
            nc.vector.memset(negc0, -C0)

            def issue_loads():
                nc.sync.dma_start(
                    out=w_q, in_=wqT.ap().rearrange("(ci k) (co m) -> k ci co m", k=128, m=128)
                )
                for t, d in ((b_q, bq), (b_k, bk)):
                    nc.sync.dma_start(out=t, in_=d.ap().rearrange("(cc c) x -> c cc x", c=128))
                nc.sync.dma_start(out=bv_sb, in_=bv.ap())
                # chunked input loads so projections can start on partial data
                x1_ap = x1s.ap().rearrange("(cc c) n -> c cc n", c=128)
                for nb in range(NQ // 1024):
                    sl = slice(nb * 1024, (nb + 1) * 1024)
                    nc.sync.dma_start(out=x1_t[:, :, sl], in_=x1_ap[:, :, sl])
                for t, d in ((w_k, wkT), (w_v, wvT)):
                    nc.sync.dma_start(
                        out=t, in_=d.ap().rearrange("(ci k) (co m) -> k ci co m", k=128, m=128)
                    )
                x2_ap = x2.ap().rearrange("(cc c) n -> c cc n", c=128)
                for nb in range(N // 512):
                    sl = slice(nb * 512, (nb + 1) * 512)
                    nc.sync.dma_start(out=x2_t[:, :, sl], in_=x2_ap[:, :, sl])

            if not full_body:
                issue_loads()

            # bv broadcast across partitions: [128, C] = ones-column x bv_row
            bvv = singles.tile([128, C], F32)

            def compute_bvv():
                bvv_ps = ps_s.tile([128, C], F32, tag="s", name="bvv_ps")
                nc.tensor.matmul(bvv_ps, lhsT=ones_jm[0:1, :], rhs=bv_sb, start=True, stop=True)
                nc.vector.tensor_copy(bvv, bvv_ps)

            if not full_body:
                compute_bvv()

            q_t = singles.tile([128, CC, NQ], F16)
            k_t = singles.tile([128, CC, N], F16)
            v_t = singles.tile([128, NJ, C], BF16)

            for _rep in range(reps):
                if full_body:
                    issue_loads()
                    compute_bvv()
                # Q first (only needs x1), then K and V^T interleaved per n-chunk
                for nb in range(NQ // 512):
                    sl = slice(nb * 512, (nb + 1) * 512)
                    for co in range(CC):
                        ps = ps_s.tile([128, 512], F32, tag="s", name="ps")
                        for ci in range(CC):
                            nc.tensor.matmul(
                                ps, lhsT=w_q[:, ci, co, :], rhs=x1_t[:, ci, sl],
                                start=(ci == 0), stop=(ci == CC - 1),
                            )
                        nc.vector.tensor_scalar_add(q_t[:, co, sl], ps, b_q[:, co, :])
                for nb in range(N // 512):
                    sl = slice(nb * 512, (nb + 1) * 512)
                    for co in range(CC):
                        ps = ps_s.tile([128, 512], F32, tag="s", name="ps")
                        for ci in range(CC):
                            nc.tensor.matmul(
                                ps, lhsT=w_k[:, ci, co, :], rhs=x2_t[:, ci, sl],
                                start=(ci == 0), stop=(ci == CC - 1),
                            )
                        nc.vector.tensor_scalar_add(k_t[:, co, sl], ps, b_k[:, co, :])
                    for jp in range(nb * 2, nb * 2 + 2):
                        ps = ps_s.tile([128, 2, C], F32, tag="s", name="ps")
                        for u in range(2):
                            jc = jp * 2 + u
                            jsl = slice(jc * 128, (jc + 1) * 128)
                            for ci in range(CC):
                                nc.tensor.matmul(
                                    ps[:, u, :], lhsT=x2_t[:, ci, jsl],
                                    rhs=w_v[:, ci, :, :],
                                    start=(ci == 0), stop=(ci == CC - 1),
                                )
                        nc.vector.tensor_add(
                            v_t[:, jp * 2:jp * 2 + 2, :], ps,
                            bvv.rearrange("p (a c) -> p a c", a=1).broadcast_to([128, 2, C]),
                        )

                # ---------------- attention ----------------
                for ib in range(IB):
                    o_ps = [
                        ps_o.tile([128, 2, 512], F32, tag=f"o{cc}", name=f"ops{cc}")
                        for cc in range(CC)
                    ]
                    acc = accp.tile([128, IBS], F32, tag="acc")
                    for jc in range(NJ):
                        jsl = slice(jc * 128, (jc + 1) * 128)
                        e_t = epool.tile([128, IBS], BF16, tag="e")
                        for h in range(2):
                            hsl = slice(h * 512, (h + 1) * 512)
                            qsl = slice(ib * IBS + h * 512, ib * IBS + (h + 1) * 512)
                            s_ps = ps_s.tile([128, 512], F32, tag="s", name=f"s{h}")
                            for ci in range(CC):
                                nc.tensor.matmul(
                                    s_ps, lhsT=k_t[:, ci, jsl], rhs=q_t[:, ci, qsl],
                                    start=(ci == 0), stop=(ci == CC - 1),
                                )
                            nc.scalar.activation(e_t[:, hsl], s_ps, EXP, bias=negc0, scale=1.0)
                            for cc in range(CC):
                                csl = slice(cc * 128, (cc + 1) * 128)
                                nc.tensor.matmul(
                                    o_ps[cc][:, h, :], lhsT=v_t[:, jc, csl], rhs=e_t[:, hsl],
                                    start=(jc == 0), stop=(jc == NJ - 1),
                                )
                        if jc == 0:
                            nc.vector.tensor_copy(acc, e_t)
                        else:
                            nc.vector.tensor_add(acc, acc, e_t)

                    # ---- epilogue: normalize + store ----
                    # ones[128,128].T @ acc = sumE broadcast across partitions;
                    # reciprocal then runs 128 lanes wide, no extra broadcast
                    rb_sb = ep1.tile([128, 2, 512], F32, tag="rb")
                    for h in range(2):
                        hsl = slice(h * 512, (h + 1) * 512)
                        rs_ps = ps_s.tile([128, 512], F32, tag="s", name=f"rs{h}")
                        nc.tensor.matmul(rs_ps, lhsT=ones_jm, rhs=acc[:, hsl],
                                         start=True, stop=True)
                        nc.vector.reciprocal(rb_sb[:, h, :], rs_ps)
                    for cc in range(CC):
                        o1 = ep2.tile([128, 2, 512], F32, tag="o1")
                        nc.vector.tensor_mul(o1, o_ps[cc], rb_sb)
                        nc.sync.dma_start(
                            out=out.ap()[cc * 128:(cc + 1) * 128,
                                         ib * IBS:(ib + 1) * IBS]
                            .rearrange("c (a b) -> c a b", a=2),
                            in_=o1,
                        )
    nc.compile()
    return nc


def kernel(x1, x2, Wq, bq, Wk, bk, Wv, bv):
    x1 = np.ascontiguousarray(np.asarray(x1, dtype=np.float32)).reshape(B, C, N).astype(np.float16)
    x2 = np.ascontiguousarray(np.asarray(x2, dtype=np.float32)).reshape(B, C, N).astype(np.float16)
    wqT = np.ascontiguousarray(np.asarray(Wq, dtype=np.float32).T).astype(np.float16)
    wkT = np.ascontiguousarray(np.asarray(Wk, dtype=np.float32).T).astype(np.float16)
    wvT = np.ascontiguousarray(np.asarray(Wv, dtype=np.float32).T).astype(np.float16)
    bq = np.asarray(bq, dtype=np.float32).reshape(C, 1)
    bk = np.asarray(bk, dtype=np.float32).reshape(C, 1)
    bv = np.asarray(bv, dtype=np.float32).reshape(1, C)

    if "nc" not in _CACHED:
        _CACHED["nc"] = _build()
    nc = _CACHED["nc"]

    in_maps = []
    for core in range(8):
        b, half = divmod(core, 2)
        in_maps.append({
            "x1s": np.ascontiguousarray(x1[b][:, half * NQ:(half + 1) * NQ]),
            "x2": x2[b],
            "wqT": wqT, "wkT": wkT, "wvT": wvT,
            "bq": bq, "bk": bk, "bv": bv,
        })
    res = run_bass_kernel_spmd(nc, in_maps, core_ids=list(range(8)))
    out = np.empty((B, C, N), dtype=np.float32)
    for core in range(8):
        b, half = divmod(core, 2)
        out[b][:, half * NQ:(half + 1) * NQ] = res.results[core]["o"]
    return out.reshape(B, C, H, W)


# revision 10
# speedup vs baseline: 1.1929x; 1.0487x over previous
"""Cross-attention (B=4, C=256, H=W=64) on 8 TRN2 NeuronCores.

Sharding: data-parallel over batch (4) x sequence-parallel over query dim
(2 halves of n=4096), one (batch, half) pair per core. Full n x n attention
stays on-core; softmax rows (over keys j) are complete locally.

Per-core math, all layouts chosen so no large transpose is ever needed:
  Q[c,i]  = WqT.T @ x1_half + bq          (fp16 matmuls, DVE bias-add)
  K[c,j]  = WkT.T @ x2 + bk
  Vt[j,c] = x2_chunk.T @ WvT + bv         (V transposed "for free", bf16;
                                           bv folded here: softmax weights
                                           sum to 1, so (V+bv)@attn adds bv
                                           to the output exactly)
  St[j,i] = K_chunk.T @ Q                 (K chunk stationary, fp16, fp32 acc)
  Et      = exp(St - C0)                  (ACT, bf16 out; C0 global shift,
                                           cancels exactly in normalization;
                                           bf16 keeps fp32 exponent range.
                                           one 1024-wide ACT op per key
                                           chunk; O/acc consumption is
                                           deferred two chunks so the ACT->PE
                                           handoff latency never gates the
                                           serial exp stream)
  O[c,i] += Vt_chunk.T @ Et               (bf16, accumulated over j in PSUM)
  sumE[i] = ones.T @ (sum_chunks Et)      (DVE accumulate + matmul, fp32)
  out     = O * (1/sumE)                  (DVE reciprocal + mul, fp32)

ACT does nothing but the 64 exps (projection bias-adds live on DVE as
tensor_scalar_add), and the E-sum accumulation is split into DVE + GpSimd
chains, so the body is bound by the exp stream (fast silicon) or raw PE
matmul throughput (throttled silicon); every other engine hides underneath.

fp16 carries the accuracy-critical S side (11-bit mantissa at full PE rate);
bf16 carries E (values up to e^79 need fp32-range exponents). All PSUM
accumulation is fp32; normalization is exact fp32.
reps>1 repeats the compute body inside one NEFF (timing harness only);
full_body additionally repeats the input DMAs each rep.
"""
import numpy as np

import concourse.bacc as bacc
import concourse.mybir as mybir
import concourse.tile as tile
from concourse.bass_utils import run_bass_kernel_spmd

B, C, H, W = 4, 256, 64, 64
N = H * W                 # 4096 keys per sample
NQ = N // 2               # 2048 queries per core
CC = C // 128             # 2 channel chunks
NJ = N // 128             # 32 key chunks
IB = 2                    # i-blocks of 1024 queries
IBS = NQ // IB            # 1024
C0 = 72.0                 # global softmax shift (see module docstring)

F32 = mybir.dt.float32
F16 = mybir.dt.float16
BF16 = mybir.dt.bfloat16
EXP = mybir.ActivationFunctionType.Exp

_CACHED = {}


def _build(reps=1, full_body=False):
    nc = bacc.Bacc()
    x1s = nc.dram_tensor("x1s", [C, NQ], F16, kind="ExternalInput")
    x2 = nc.dram_tensor("x2", [C, N], F16, kind="ExternalInput")
    wqT = nc.dram_tensor("wqT", [C, C], F16, kind="ExternalInput")
    wkT = nc.dram_tensor("wkT", [C, C], F16, kind="ExternalInput")
    wvT = nc.dram_tensor("wvT", [C, C], F16, kind="ExternalInput")
    bq = nc.dram_tensor("bq", [C, 1], F32, kind="ExternalInput")
    bk = nc.dram_tensor("bk", [C, 1], F32, kind="ExternalInput")
    bv = nc.dram_tensor("bv", [1, C], F32, kind="ExternalInput")
    out = nc.dram_tensor("o", [C, NQ], F32, kind="ExternalOutput")

    with tile.TileContext(nc) as tc:
        with (
            tc.tile_pool(name="singles", bufs=1) as singles,
            tc.tile_pool(name="epool", bufs=4) as epool,
            tc.tile_pool(name="accp", bufs=2) as accp,
            tc.tile_pool(name="ep1", bufs=2) as ep1,
            tc.tile_pool(name="ep2", bufs=2) as ep2,
            tc.tile_pool(name="ps_s", bufs=2, space="PSUM") as ps_s,  # 2x 2-bank
            tc.tile_pool(name="ps_o", bufs=1, space="PSUM") as ps_o,  # 4 banks
        ):
            # ---------------- tiles + constants ----------------
            w_q = singles.tile([128, CC, CC, 128], F16)
            w_k = singles.tile([128, CC, CC, 128], F16)
            w_v = singles.tile([128, CC, CC, 128], F16)
            b_q = singles.tile([128, CC, 1], F32)
            b_k = singles.tile([128, CC, 1], F32)
            bv_sb = singles.tile([1, C], F32)
            x1_t = singles.tile([128, CC, NQ], F16)
            x2_t = singles.tile([128, CC, N], F16)

            ones_jm = singles.tile([128, 128], F32)
            nc.vector.memset(ones_jm, 1.0)
            negc0 = singles.tile([128, 1], F32)
            nc.vector.memset(negc0, -C0)

            def issue_loads():
                nc.sync.dma_start(
                    out=w_q, in_=wqT.ap().rearrange("(ci k) (co m) -> k ci co m", k=128, m=128)
                )
                for t, d in ((b_q, bq), (b_k, bk)):
                    nc.sync.dma_start(out=t, in_=d.ap().rearrange("(cc c) x -> c cc x", c=128))
                nc.sync.dma_start(out=bv_sb, in_=bv.ap())
                # chunked input loads so projections can start on partial data
                x1_ap = x1s.ap().rearrange("(cc c) n -> c cc n", c=128)
                for nb in range(NQ // 1024):
                    sl = slice(nb * 1024, (nb + 1) * 1024)
                    nc.sync.dma_start(out=x1_t[:, :, sl], in_=x1_ap[:, :, sl])
                for t, d in ((w_k, wkT), (w_v, wvT)):
                    nc.sync.dma_start(
                        out=t, in_=d.ap().rearrange("(ci k) (co m) -> k ci co m", k=128, m=128)
                    )
                x2_ap = x2.ap().rearrange("(cc c) n -> c cc n", c=128)
                for nb in range(N // 512):
                    sl = slice(nb * 512, (nb + 1) * 512)
                    nc.sync.dma_start(out=x2_t[:, :, sl], in_=x2_ap[:, :, sl])

            if not full_body:
                issue_loads()

            # bv broadcast across partitions: [128, C] = ones-column x bv_row
            bvv = singles.tile([128, C], F32)

            def compute_bvv():
                bvv_ps = ps_s.tile([128, C], F32, tag="s", name="bvv_ps")
                nc.tensor.matmul(bvv_ps, lhsT=ones_jm[0:1, :], rhs=bv_sb, start=True, stop=True)
                nc.vector.tensor_copy(bvv, bvv_ps)

            if not full_body:
                compute_bvv()

            q_t = singles.tile([128, CC, NQ], F16)
            k_t = singles.tile([128, CC, N], F16)
            v_t = singles.tile([128, NJ, C], BF16)

            for _rep in range(reps):
                if full_body:
                    issue_loads()
                    compute_bvv()
                # Q first (only needs x1), then K and V^T interleaved per n-chunk
                for nb in range(NQ // 512):
                    sl = slice(nb * 512, (nb + 1) * 512)
                    for co in range(CC):
                        ps = ps_s.tile([128, 512], F32, tag="s", name="ps")
                        for ci in range(CC):
                            nc.tensor.matmul(
                                ps, lhsT=w_q[:, ci, co, :], rhs=x1_t[:, ci, sl],
                                start=(ci == 0), stop=(ci == CC - 1),
                            )
                        nc.vector.tensor_scalar_add(q_t[:, co, sl], ps, b_q[:, co, :])
                for nb in range(N // 512):
                    sl = slice(nb * 512, (nb + 1) * 512)
                    for co in range(CC):
                        ps = ps_s.tile([128, 512], F32, tag="s", name="ps")
                        for ci in range(CC):
                            nc.tensor.matmul(
                                ps, lhsT=w_k[:, ci, co, :], rhs=x2_t[:, ci, sl],
                                start=(ci == 0), stop=(ci == CC - 1),
                            )
                        nc.vector.tensor_scalar_add(k_t[:, co, sl], ps, b_k[:, co, :])
                    for jp in range(nb * 2, nb * 2 + 2):
                        ps = ps_s.tile([128, 2, C], F32, tag="s", name="ps")
                        for u in range(2):
                            jc = jp * 2 + u
                            jsl = slice(jc * 128, (jc + 1) * 128)
                            for ci in range(CC):
                                nc.tensor.matmul(
                                    ps[:, u, :], lhsT=x2_t[:, ci, jsl],
                                    rhs=w_v[:, ci, :, :],
                                    start=(ci == 0), stop=(ci == CC - 1),
                                )
                        nc.vector.tensor_add(
                            v_t[:, jp * 2:jp * 2 + 2, :], ps,
                            bvv.rearrange("p (a c) -> p a c", a=1).broadcast_to([128, 2, C]),
                        )

                # ---------------- attention ----------------
                for ib in range(IB):
                    o_ps = [
                        ps_o.tile([128, 2, 512], F32, tag=f"o{cc}", name=f"ops{cc}")
                        for cc in range(CC)
                    ]
                    # E-sum split into two chains (DVE + GpSimd) so the
                    # serial accumulate never binds either engine
                    acc_d = accp.tile([128, IBS], F32, tag="accd")
                    acc_g = accp.tile([128, IBS], F32, tag="accg")
                    pend = []  # (jc, e_t) with O/acc deferred 2 steps: O[jc-2]
                    # only needs an exp that finished two ACT periods ago, so
                    # the ACT->PE handoff latency never gates the exp stream

                    def emit_o_acc(jcp, e):
                        for cc in range(CC):
                            csl = slice(cc * 128, (cc + 1) * 128)
                            for h in range(2):
                                hsl = slice(h * 512, (h + 1) * 512)
                                nc.tensor.matmul(
                                    o_ps[cc][:, h, :], lhsT=v_t[:, jcp, csl],
                                    rhs=e[:, hsl],
                                    start=(jcp == 0), stop=(jcp == NJ - 1),
                                )
                        if jcp == 0:
                            nc.vector.tensor_copy(acc_d, e)
                        elif jcp == 1:
                            nc.gpsimd.tensor_copy(acc_g, e)
                        elif jcp % 2 == 0:
                            nc.vector.tensor_add(acc_d, acc_d, e)
                        else:
                            nc.gpsimd.tensor_add(acc_g, acc_g, e)

                    for jc in range(NJ):
                        jsl = slice(jc * 128, (jc + 1) * 128)
                        s_ps = ps_s.tile([128, IBS], F32, tag="s", name="s_ps")
                        for ci in range(CC):
                            for h in range(2):
                                hsl = slice(h * 512, (h + 1) * 512)
                                qsl = slice(ib * IBS + h * 512, ib * IBS + (h + 1) * 512)
                                nc.tensor.matmul(
                                    s_ps[:, hsl], lhsT=k_t[:, ci, jsl], rhs=q_t[:, ci, qsl],
                                    start=(ci == 0), stop=(ci == CC - 1),
                                )
                        if len(pend) == 2:
                            emit_o_acc(*pend.pop(0))
                        e_t = epool.tile([128, IBS], BF16, tag="e")
                        nc.scalar.activation(e_t, s_ps, EXP, bias=negc0, scale=1.0)
                        pend.append((jc, e_t))
                    for p in pend:
                        emit_o_acc(*p)

                    # ---- epilogue: normalize + store ----
                    # ones[128,128].T @ acc = sumE broadcast across partitions;
                    # reciprocal then runs 128 lanes wide, no extra broadcast
                    acc = accp.tile([128, IBS], F32, tag="acc")
                    nc.vector.tensor_add(acc, acc_d, acc_g)
                    rb_sb = ep1.tile([128, 2, 512], F32, tag="rb")
                    for h in range(2):
                        hsl = slice(h * 512, (h + 1) * 512)
                        rs_ps = ps_s.tile([128, 512], F32, tag="s", name=f"rs{h}")
                        nc.tensor.matmul(rs_ps, lhsT=ones_jm, rhs=acc[:, hsl],
                                         start=True, stop=True)
                        nc.vector.reciprocal(rb_sb[:, h, :], rs_ps)
                    for cc in range(CC):
                        o1 = ep2.tile([128, 2, 512], F32, tag="o1")
                        nc.vector.tensor_mul(o1, o_ps[cc], rb_sb)
                        nc.sync.dma_start(
                            out=out.ap()[cc * 128:(cc + 1) * 128,
                                         ib * IBS:(ib + 1) * IBS]
                            .rearrange("c (a b) -> c a b", a=2),
                            in_=o1,
                        )
    nc.compile()
    return nc


def kernel(x1, x2, Wq, bq, Wk, bk, Wv, bv):
    x1 = np.ascontiguousarray(np.asarray(x1, dtype=np.float32)).reshape(B, C, N).astype(np.float16)
    x2 = np.ascontiguousarray(np.asarray(x2, dtype=np.float32)).reshape(B, C, N).astype(np.float16)
    wqT = np.ascontiguousarray(np.asarray(Wq, dtype=np.float32).T).astype(np.float16)
    wkT = np.ascontiguousarray(np.asarray(Wk, dtype=np.float32).T).astype(np.float16)
    wvT = np.ascontiguousarray(np.asarray(Wv, dtype=np.float32).T).astype(np.float16)
    bq = np.asarray(bq, dtype=np.float32).reshape(C, 1)
    bk = np.asarray(bk, dtype=np.float32).reshape(C, 1)
    bv = np.asarray(bv, dtype=np.float32).reshape(1, C)

    if "nc" not in _CACHED:
        _CACHED["nc"] = _build()
    nc = _CACHED["nc"]

    in_maps = []
    for core in range(8):
        b, half = divmod(core, 2)
        in_maps.append({
            "x1s": np.ascontiguousarray(x1[b][:, half * NQ:(half + 1) * NQ]),
            "x2": x2[b],
            "wqT": wqT, "wkT": wkT, "wvT": wvT,
            "bq": bq, "bk": bk, "bv": bv,
        })
    res = run_bass_kernel_spmd(nc, in_maps, core_ids=list(range(8)))
    out = np.empty((B, C, N), dtype=np.float32)
    for core in range(8):
        b, half = divmod(core, 2)
        out[b][:, half * NQ:(half + 1) * NQ] = res.results[core]["o"]
    return out.reshape(B, C, H, W)


# revision 13
# speedup vs baseline: 1.2003x; 1.0062x over previous
"""Cross-attention (B=4, C=256, H=W=64) on 8 TRN2 NeuronCores.

Sharding: data-parallel over batch (4) x sequence-parallel over query dim
(2 halves of n=4096), one (batch, half) pair per core. Full n x n attention
stays on-core; softmax rows (over keys j) are complete locally.

Per-core math, all layouts chosen so no large transpose is ever needed:
  QW[u,i] = (Wk^T Wq) @ x1_half + Wk^T bq (fp16 matmuls, DVE bias-add;
                                           the K projection is FUSED into the
                                           query side: S = K^T Q = X2^T QW up
                                           to a per-query logit offset bk.Q
                                           that cancels in softmax. W2 and
                                           Wk^T bq are precomputed on host)
  Vt[j,c] = x2_chunk.T @ WvT + bv         (V transposed "for free", bf16;
                                           bv folded here: softmax weights
                                           sum to 1, so (V+bv)@attn adds bv
                                           to the output exactly)
  St[j,i] = X2_chunk.T @ QW               (x2 chunk stationary, fp16, f32 acc)
  Et      = exp(St - C0)                  (ACT, bf16 out; C0 global shift,
                                           cancels exactly in normalization;
                                           bf16 keeps fp32 exponent range.
                                           one 1024-wide ACT op per key
                                           chunk; O/acc consumption is
                                           deferred two chunks so the ACT->PE
                                           handoff latency never gates the
                                           serial exp stream)
  O[c,i] += Vt_chunk.T @ Et               (bf16, accumulated over j in PSUM)
  sumE[i] = ones.T @ (sum_chunks Et)      (DVE accumulate + matmul, fp32)
  out     = O * (1/sumE)                  (DVE reciprocal + mul, fp32)

ACT does nothing but the 64 exps (projection bias-adds live on DVE as
tensor_scalar_add), and the E-sum accumulation is split into DVE + GpSimd
chains, so the body is bound by the exp stream (fast silicon) or raw PE
matmul throughput (throttled silicon); every other engine hides underneath.

fp16 carries the accuracy-critical S side (11-bit mantissa at full PE rate);
bf16 carries E (values up to e^79 need fp32-range exponents). All PSUM
accumulation is fp32; normalization is exact fp32.
reps>1 repeats the compute body inside one NEFF (timing harness only);
full_body additionally repeats the input DMAs each rep.
"""
import numpy as np

import concourse.bacc as bacc
import concourse.mybir as mybir
import concourse.tile as tile
from concourse.bass_utils import run_bass_kernel_spmd

B, C, H, W = 4, 256, 64, 64
N = H * W                 # 4096 keys per sample
NQ = N // 2               # 2048 queries per core
CC = C // 128             # 2 channel chunks
NJ = N // 128             # 32 key chunks
IB = 2                    # i-blocks of 1024 queries
IBS = NQ // IB            # 1024
C0 = 72.0                 # global softmax shift (see module docstring)

F32 = mybir.dt.float32
F16 = mybir.dt.float16
BF16 = mybir.dt.bfloat16
EXP = mybir.ActivationFunctionType.Exp
IDENT = mybir.ActivationFunctionType.Identity

_CACHED = {}


def _build(reps=1, full_body=False):
    nc = bacc.Bacc()
    x1s = nc.dram_tensor("x1s", [C, NQ], F16, kind="ExternalInput")
    x2 = nc.dram_tensor("x2", [C, N], F16, kind="ExternalInput")
    w2T = nc.dram_tensor("w2T", [C, C], F16, kind="ExternalInput")
    wvT = nc.dram_tensor("wvT", [C, C], BF16, kind="ExternalInput")
    qwb = nc.dram_tensor("qwb", [C, 1], F32, kind="ExternalInput")
    bv = nc.dram_tensor("bv", [1, C], F32, kind="ExternalInput")
    out = nc.dram_tensor("o", [C, NQ], F32, kind="ExternalOutput")

    with tile.TileContext(nc) as tc:
        with (
            tc.tile_pool(name="singles", bufs=1) as singles,
            tc.tile_pool(name="epool", bufs=4) as epool,
            tc.tile_pool(name="accp", bufs=2) as accp,
            tc.tile_pool(name="ep1", bufs=2) as ep1,
            tc.tile_pool(name="ep2", bufs=2) as ep2,
            tc.tile_pool(name="ps_s", bufs=2, space="PSUM") as ps_s,  # 2x 2-bank
            tc.tile_pool(name="ps_o", bufs=1, space="PSUM") as ps_o,  # 4 banks
        ):
            # ---------------- tiles + constants ----------------
            w_2 = singles.tile([128, CC, CC, 128], F16)
            w_v = singles.tile([128, CC, CC, 128], BF16)
            b_qw = singles.tile([128, CC, 1], F32)
            bv_sb = singles.tile([1, C], F32)
            x1_t = singles.tile([128, CC, NQ], F16)
            x2_t = singles.tile([128, CC, N], F16)

            ones_jm = singles.tile([128, 128], F32)
            nc.vector.memset(ones_jm, 1.0)
            negc0 = singles.tile([128, 1], F32)
            nc.vector.memset(negc0, -C0)

            # warm the ACT exp table set during the DMA/projection lead-in:
            # depends only on the memset, so it issues immediately and the
            # ~2.7us PSEUDO_LOAD_ACT_FUNC_SET never lands on the first real exp
            exp_warm = singles.tile([128, 1], F32)
            nc.scalar.activation(exp_warm, negc0, EXP, bias=0.0, scale=1.0)

            def issue_loads():
                nc.sync.dma_start(
                    out=w_2, in_=w2T.ap().rearrange("(ci k) (co m) -> k ci co m", k=128, m=128)
                )
                nc.sync.dma_start(out=b_qw, in_=qwb.ap().rearrange("(cc c) x -> c cc x", c=128))
                nc.sync.dma_start(out=bv_sb, in_=bv.ap())
                # chunked input loads so projections can start on partial data
                x1_ap = x1s.ap().rearrange("(cc c) n -> c cc n", c=128)
                for nb in range(NQ // 1024):
                    sl = slice(nb * 1024, (nb + 1) * 1024)
                    nc.sync.dma_start(out=x1_t[:, :, sl], in_=x1_ap[:, :, sl])
                nc.sync.dma_start(
                    out=w_v, in_=wvT.ap().rearrange("(ci k) (co m) -> k ci co m", k=128, m=128)
                )
                x2_ap = x2.ap().rearrange("(cc c) n -> c cc n", c=128)
                for nb in range(N // 512):
                    sl = slice(nb * 512, (nb + 1) * 512)
                    nc.sync.dma_start(out=x2_t[:, :, sl], in_=x2_ap[:, :, sl])

            if not full_body:
                issue_loads()

            # bv broadcast across partitions: [128, C] = ones-column x bv_row
            bvv = singles.tile([128, C], F32)

            def compute_bvv():
                bvv_ps = ps_s.tile([128, C], F32, tag="s", name="bvv_ps")
                nc.tensor.matmul(bvv_ps, lhsT=ones_jm[0:1, :], rhs=bv_sb, start=True, stop=True)
                nc.vector.tensor_copy(bvv, bvv_ps)

            if not full_body:
                compute_bvv()

            qw_t = singles.tile([128, CC, NQ], F16)
            v_t = singles.tile([128, NJ, C], BF16)

            for _rep in range(reps):
                if full_body:
                    issue_loads()
                    compute_bvv()
                # QW = (Wk^T Wq) X1 + Wk^T bq: K-projection is fused into the
                # query side (host precomputes W2); the per-query bk.Q softmax
                # logit offset is j-independent and cancels in normalization
                for nb in range(NQ // 512):
                    sl = slice(nb * 512, (nb + 1) * 512)
                    for co in range(CC):
                        ps = ps_s.tile([128, 512], F32, tag="s", name="ps")
                        for ci in range(CC):
                            nc.tensor.matmul(
                                ps, lhsT=w_2[:, ci, co, :], rhs=x1_t[:, ci, sl],
                                start=(ci == 0), stop=(ci == CC - 1),
                            )
                        if co == 0:
                            nc.vector.tensor_scalar_add(qw_t[:, co, sl], ps, b_qw[:, co, :])
                        else:
                            nc.scalar.activation(qw_t[:, co, sl], ps, IDENT, bias=b_qw[:, co, :])
                for nb in range(N // 512):
                    sl = slice(nb * 512, (nb + 1) * 512)
                    for jp in range(nb * 2, nb * 2 + 2):
                        ps = ps_s.tile([128, 2, C], F32, tag="s", name="ps")
                        for u in range(2):
                            jc = jp * 2 + u
                            jsl = slice(jc * 128, (jc + 1) * 128)
                            for ci in range(CC):
                                nc.tensor.matmul(
                                    ps[:, u, :], lhsT=x2_t[:, ci, jsl],
                                    rhs=w_v[:, ci, :, :],
                                    start=(ci == 0), stop=(ci == CC - 1),
                                )
                        nc.vector.tensor_add(
                            v_t[:, jp * 2:jp * 2 + 2, :], ps,
                            bvv.rearrange("p (a c) -> p a c", a=1).broadcast_to([128, 2, C]),
                        )

                # ---------------- attention ----------------
                for ib in range(IB):
                    o_ps = [
                        ps_o.tile([128, 2, 512], F32, tag=f"o{cc}", name=f"ops{cc}")
                        for cc in range(CC)
                    ]
                    # E-sum split into two chains (DVE + GpSimd) so the
                    # serial accumulate never binds either engine
                    acc_d = accp.tile([128, IBS], F32, tag="accd")
                    acc_g = accp.tile([128, IBS], F32, tag="accg")
                    pend = []  # (jc, e_t) with O/acc deferred 2 steps: O[jc-2]
                    # only needs an exp that finished two ACT periods ago, so
                    # the ACT->PE handoff latency never gates the exp stream

                    def emit_o_acc(jcp, e):
                        for cc in range(CC):
                            csl = slice(cc * 128, (cc + 1) * 128)
                            for h in range(2):
                                hsl = slice(h * 512, (h + 1) * 512)
                                nc.tensor.matmul(
                                    o_ps[cc][:, h, :], lhsT=v_t[:, jcp, csl],
                                    rhs=e[:, hsl],
                                    start=(jcp == 0), stop=(jcp == NJ - 1),
                                )
                        if jcp == 0:
                            nc.vector.tensor_copy(acc_d, e)
                        elif jcp == 1:
                            nc.gpsimd.tensor_copy(acc_g, e)
                        elif jcp % 2 == 0:
                            nc.vector.tensor_add(acc_d, acc_d, e)
                        else:
                            nc.gpsimd.tensor_add(acc_g, acc_g, e)

                    for jc in range(NJ):
                        jsl = slice(jc * 128, (jc + 1) * 128)
                        s_ps = ps_s.tile([128, IBS], F32, tag="s", name="s_ps")
                        for ci in range(CC):
                            for h in range(2):
                                hsl = slice(h * 512, (h + 1) * 512)
                                qsl = slice(ib * IBS + h * 512, ib * IBS + (h + 1) * 512)
                                nc.tensor.matmul(
                                    s_ps[:, hsl], lhsT=x2_t[:, ci, jsl], rhs=qw_t[:, ci, qsl],
                                    start=(ci == 0), stop=(ci == CC - 1),
                                )
                        if len(pend) == 2:
                            emit_o_acc(*pend.pop(0))
                        e_t = epool.tile([128, IBS], BF16, tag="e")
                        nc.scalar.activation(e_t, s_ps, EXP, bias=negc0, scale=1.0)
                        pend.append((jc, e_t))
                    for p in pend:
                        emit_o_acc(*p)

                    # ---- epilogue: normalize + store ----
                    # ones[128,128].T @ acc = sumE broadcast across partitions;
                    # reciprocal then runs 128 lanes wide, no extra broadcast
                    acc = accp.tile([128, IBS], F32, tag="acc")
                    nc.vector.tensor_add(acc, acc_d, acc_g)
                    rb_sb = ep1.tile([128, 2, 512], F32, tag="rb")
                    for h in range(2):
                        hsl = slice(h * 512, (h + 1) * 512)
                        rs_ps = ps_s.tile([128, 512], F32, tag="s", name=f"rs{h}")
                        nc.tensor.matmul(rs_ps, lhsT=ones_jm, rhs=acc[:, hsl],
                                         start=True, stop=True)
                        nc.vector.reciprocal(rb_sb[:, h, :], rs_ps)
                    for cc in range(CC):
                        o1 = ep2.tile([128, 2, 512], F32, tag="o1")
                        nc.vector.tensor_mul(o1, o_ps[cc], rb_sb)
                        nc.sync.dma_start(
                            out=out.ap()[cc * 128:(cc + 1) * 128,
                                         ib * IBS:(ib + 1) * IBS]
                            .rearrange("c (a b) -> c a b", a=2),
                            in_=o1,
                        )
    nc.compile()
    return nc


def kernel(x1, x2, Wq, bq, Wk, bk, Wv, bv):
    x1 = np.ascontiguousarray(np.asarray(x1, dtype=np.float32)).reshape(B, C, N).astype(np.float16)
    x2 = np.ascontiguousarray(np.asarray(x2, dtype=np.float32)).reshape(B, C, N).astype(np.float16)
    Wqf = np.asarray(Wq, dtype=np.float32)
    Wkf = np.asarray(Wk, dtype=np.float32)
    w2T = np.ascontiguousarray(Wqf.T @ Wkf).astype(np.float16)
    import ml_dtypes
    wvT = np.ascontiguousarray(np.asarray(Wv, dtype=np.float32).T).astype(ml_dtypes.bfloat16)
    qwb = (Wkf.T @ np.asarray(bq, dtype=np.float32).reshape(C)).reshape(C, 1)
    bv = np.asarray(bv, dtype=np.float32).reshape(1, C)

    if "nc" not in _CACHED:
        _CACHED["nc"] = _build()
    nc = _CACHED["nc"]

    in_maps = []
    for core in range(8):
        b, half = divmod(core, 2)
        in_maps.append({
            "x1s": np.ascontiguousarray(x1[b][:, half * NQ:(half + 1) * NQ]),
            "x2": x2[b],
            "w2T": w2T, "wvT": wvT, "qwb": qwb, "bv": bv,
        })
    res = run_bass_kernel_spmd(nc, in_maps, core_ids=list(range(8)))
    out = np.empty((B, C, N), dtype=np.float32)
    for core in range(8):
        b, half = divmod(core, 2)
        out[b][:, half * NQ:(half + 1) * NQ] = res.results[core]["o"]
    return out.reshape(B, C, H, W)
